# revision 29
# baseline (speedup 1.0000x reference)
"""CARAFE content-aware upsampling kernel for Trainium2 (Bass/Tile), 8 NeuronCores.

Problem (hardcoded): features [4, 256, 64, 64] f32, masks [4, 25, 128, 128] f32,
K=5, G=1, S=2 -> output [4, 256, 128, 128] f32.

Strategy
--------
Sharding: 8 cores = (batch n in 0..3) x (output-row half yh in 0..1); each core
computes out[n, :, yh*64:(yh+1)*64, :] for all 256 channels.

Compute mapping: the 25-tap weighted sum for a block of output pixels is cast
as one PSUM accumulation group of TensorEngine matmuls whose contraction axis
packs (feature row rl, padded column wl) pairs:

  block (bg, c) covers pixels (pair l2 = 4*bg+pl, py, x = 16*c+xl).
  Receptive field: padded rows hp = 4*bg..4*bg+7 (row chunks j = bg, bg+1 of
  4 rows each, shared with the neighbor blocks), cols wp = 8*c..8*c+11.

  psum[ch, (pl py xl)] += sum_{rl, wl} F[j][c][(rl wl), ch] * B[(rl wl), (py xl)]

F[j][c] = ft[4j+rl, 8c+wl, ch] (48 x 256 bf16, host-packed). The mask operand
for pixel row pl only has nonzero contraction rows where the row tap
kr = rl - pl (chunk j = bg) resp. 4 + rl - pl (chunk bg+1) lies in [0, 5), so
each (bg, c, ch-half, c-quad) is computed by 6 matmuls whose partition ranges
skip most structural zeros (operand base partitions must be 0/32/64, counts
are free):

  X-main: rows [0, 48) of chunk bg,   pixel rows pl 0..2 (96 cols)
  X3:     rows [32, 48) of chunk bg,  pl 3
  Y(pl):  rows [0, 12*(pl+1)) of chunk bg+1, one per pl

The host packs exactly those rectangles (1.15 MB/core vs 3.1 MB dense) into
five HBM sections (G = full-height X-main+Y3 strips, B/C/D/E = partial-height
X3/Y0/Y1/Y2 strips), each with >= 512B contiguous runs so DMA runs at the
full 360 GB/s.

Per (bg, ch-half, c-quad) one PSUM bank tile [128, 512] takes 24 matmuls,
start=True once per bank (clears the whole bank's has_written bits), stop=True
on the last. DVE/ACT copy+cast each bank into a [128, 2048] bf16 staging tile
per bg; per-(bg, ch) DMAs write 8 output rows x 128 channels (2KB contiguous
runs). Junk warm-up matmuls on a memset SBUF tile ride out the PE p-state ramp
while the first input DMAs are in flight. The host upcasts bf16 to f32.
"""

import sys

sys.path.insert(0, "/opt/trn_rl_repo")

import numpy as np
import ml_dtypes

import concourse.bacc as bacc
import concourse.mybir as mybir
from concourse import tile
from concourse import bass_utils

N, C, H, W = 4, 256, 64, 64
S = 2
KK = 5
HO, WO = H * S, W * S  # 128, 128
NCORES = 8

NBG = 8   # row-pair groups per core (4 pairs = 8 output rows each)
NCH = 8   # x chunks per core (16 output cols each)
NJ = 9    # 4-row feature chunks per core (36 padded rows)
RW = 48   # contraction partitions per block: 4 rows x 12 wl
FTF = NJ * NCH * C  # 18432 ftb free elems
BNF = 16384         # bnd tile free elems (G 8192 + B/C/D/E 2048 each)

BF16 = ml_dtypes.bfloat16

# bnd tile free-column offsets
GOFF, BOFF, COFF, DOFF, EOFF = 0, 8192, 10240, 12288, 14336


def _bnd_dense():
    """Index arrays for the dense banded masks [bg, xy, c, rl, wl, pl, py, xl]."""
    bg = np.arange(NBG).reshape(NBG, 1, 1, 1, 1, 1, 1, 1)
    xy = np.arange(2).reshape(1, 2, 1, 1, 1, 1, 1, 1)
    c = np.arange(NCH).reshape(1, 1, NCH, 1, 1, 1, 1, 1)
    rl = np.arange(4).reshape(1, 1, 1, 4, 1, 1, 1, 1)
    wl = np.arange(12).reshape(1, 1, 1, 1, 12, 1, 1, 1)
    pl = np.arange(4).reshape(1, 1, 1, 1, 1, 4, 1, 1)
    py = np.arange(2).reshape(1, 1, 1, 1, 1, 1, 2, 1)
    xl = np.arange(16).reshape(1, 1, 1, 1, 1, 1, 1, 16)
    kr = rl - pl + 4 * xy
    dw = wl - xl // 2
    valid = (kr >= 0) & (kr <= 4) & (dw >= 0) & (dw <= 4)
    chan = np.clip(kr, 0, 4) * KK + np.clip(dw, 0, 4)
    ylo = 8 * bg + 2 * pl + py
    x = 16 * c + xl
    return np.broadcast_arrays(chan, ylo, x, valid)


_CHAN, _YLO, _X, _VALID = _bnd_dense()


def _host_prep(features: np.ndarray, masks: np.ndarray):
    """Per-core packed feature chunks and trimmed banded mask sections."""
    ftg = np.zeros((N, H + 4, W + 4, C), np.float32)
    ftg[:, 2 : 2 + H, 2 : 2 + W, :] = features.transpose(0, 2, 3, 1)

    maps = []
    for i in range(NCORES):
        n, yh = divmod(i, 2)
        flp = ftg[n, 32 * yh : 32 * yh + 36]  # [36, 68, C]
        fj = flp.reshape(NJ, 4, W + 4, C)
        s = fj.strides
        fw = np.lib.stride_tricks.as_strided(
            fj, shape=(NJ, 4, NCH, 12, C), strides=(s[0], s[1], 8 * s[2], s[2], s[3])
        )
        ftb = np.ascontiguousarray(fw.transpose(1, 3, 0, 2, 4)).reshape(RW, FTF)

        m = masks[n, :, 64 * yh : 64 * yh + 64, :]
        dense = np.where(_VALID, m[_CHAN, _YLO, _X], np.float32(0.0))
        # [bg, xy, c, rl, wl, pl, py, xl] -> [rw, bg, xy, c, pl, py*xl]
        d6 = dense.transpose(3, 4, 0, 1, 2, 5, 6, 7).reshape(RW, NBG, 2, NCH, 4, 32)
        g = np.empty((RW, NBG, NCH, 4, 32), np.float32)
        g[:, :, :, 0:3, :] = d6[:, :, 0, :, 0:3, :]  # X-main (pl 0..2)
        g[:, :, :, 3, :] = d6[:, :, 1, :, 3, :]      # Y3
        maps.append({
            "ftb": ftb.astype(BF16),
            "bndG": np.ascontiguousarray(g).reshape(RW, 8192).astype(BF16),
            "bndB": np.ascontiguousarray(d6[:, :, 0, :, 3, :]).reshape(48, 2048).astype(BF16),
            "bndC": np.ascontiguousarray(d6[0:12, :, 1, :, 0, :]).reshape(12, 2048).astype(BF16),
            "bndD": np.ascontiguousarray(d6[0:24, :, 1, :, 1, :]).reshape(24, 2048).astype(BF16),
            "bndE": np.ascontiguousarray(d6[0:36, :, 1, :, 2, :]).reshape(36, 2048).astype(BF16),
        })
    return maps


_NC_CACHE = []


def _build_nc():
    """Build + compile the single-core Tile program (same for all 8 cores)."""
    if _NC_CACHE:
        return _NC_CACHE[0]

    nc = bacc.Bacc("TRN2", target_bir_lowering=False, debug=False)
    ftb = nc.dram_tensor("ftb", [RW, FTF], mybir.dt.bfloat16, kind="ExternalInput").ap()
    bG = nc.dram_tensor("bndG", [RW, 8192], mybir.dt.bfloat16, kind="ExternalInput").ap()
    bB = nc.dram_tensor("bndB", [48, 2048], mybir.dt.bfloat16, kind="ExternalInput").ap()
    bC = nc.dram_tensor("bndC", [12, 2048], mybir.dt.bfloat16, kind="ExternalInput").ap()
    bD = nc.dram_tensor("bndD", [24, 2048], mybir.dt.bfloat16, kind="ExternalInput").ap()
    bE = nc.dram_tensor("bndE", [36, 2048], mybir.dt.bfloat16, kind="ExternalInput").ap()
    out = nc.dram_tensor("out", [C, HO // 2 * WO], mybir.dt.bfloat16, kind="ExternalOutput").ap()
    ov = out.rearrange("(g p) f -> p g f", g=2)  # [128, 2, 8192]

    with tile.TileContext(nc) as tc:
        with (
            tc.tile_pool(name="wup", bufs=1) as wup,
            tc.tile_pool(name="ftp", bufs=1) as ftp,
            tc.tile_pool(name="bnp", bufs=1) as bnp,
            tc.tile_pool(name="pp", bufs=8, space="PSUM") as pp,
            tc.tile_pool(name="stp", bufs=5) as stp,
        ):
            # PE p-state warm-up: junk matmuls on a zeroed SBUF tile (result
            # never read) keep the tensor engine busy while the first input
            # DMAs are in flight, so real matmuls run at the 2.4 GHz p-state.
            wt = wup.tile([RW, 128], mybir.dt.bfloat16)
            nc.gpsimd.memset(wt[:], 0.0)
            wps = pp.tile([128, 128], mybir.dt.float32, name="wps", tag="ps")
            for _ in range(36):
                nc.tensor.matmul(wps[:], wt[:], wt[:], start=True, stop=True)

            ft = ftp.tile([RW, FTF], mybir.dt.bfloat16)
            bn = bnp.tile([RW, BNF], mybir.dt.bfloat16)
            # Input DMAs, interleaved so bg_k's operands land just in time
            # while keeping the HWDGE prep queue (~625ns/DMA) ahead of the
            # transfer stream.
            # C/D go through the Pool SWDGE path: their descriptor prep runs
            # on the Pool engine, in parallel with HWDGE prepping the larger
            # SP-issued DMAs, so their short transfers slot into the stream
            # without starving the DMA engines.
            nc.gpsimd.dma_start(bn[0:12, COFF : COFF + 2048], bC)
            nc.gpsimd.dma_start(bn[0:24, DOFF : DOFF + 2048], bD)
            nc.sync.dma_start(ft[:, 0:4096], ftb[:, 0:4096])              # j 0-1
            nc.sync.dma_start(bn[:, GOFF : GOFF + 2048], bG[:, 0:2048])  # G bg0-1
            nc.sync.dma_start(bn[:, BOFF : BOFF + 2048], bB)
            nc.sync.dma_start(bn[0:36, EOFF : EOFF + 2048], bE)
            nc.sync.dma_start(ft[:, 4096:8192], ftb[:, 4096:8192])       # j 2-3
            nc.sync.dma_start(bn[:, GOFF + 2048 : GOFF + 4096], bG[:, 2048:4096])  # G bg2-3
            nc.sync.dma_start(ft[:, 8192:12288], ftb[:, 8192:12288])     # j 4-5
            nc.sync.dma_start(bn[:, GOFF + 4096 : GOFF + 6144], bG[:, 4096:6144])  # G bg4-5
            nc.sync.dma_start(ft[:, 12288:18432], ftb[:, 12288:18432])   # j 6-8
            nc.sync.dma_start(bn[:, GOFF + 6144 : GOFF + 8192], bG[:, 6144:8192])  # G bg6-7

            for bg in range(NBG):
                st = stp.tile([128, 2 * 8 * WO], mybir.dt.bfloat16, name="st", tag="st")
                # st free layout: (ch2, y = 2*pl+py: 8, x = 64*half+16*cq+xl: 128)
                stv = st.rearrange(
                    "p (ch pl py xh xx) -> p ch pl py xh xx", ch=2, pl=4, py=2, xh=2
                )
                for ch in range(2):
                    for half in range(2):
                        ps = pp.tile([128, 512], mybir.dt.float32, name="ps", tag="ps")
                        # psum free layout: (pl, py, cq, xl)
                        psv = ps.rearrange("p (pl py cq xl) -> p pl py cq xl",
                                           pl=4, py=2, cq=4)
                        # X-main first: they only need ft + section G.
                        for cq in range(4):
                            ci = half * 4 + cq
                            fo = (bg * NCH + ci) * C + ch * 128
                            nc.tensor.matmul(
                                psv[:, 0:3, :, cq, :],
                                ft[:, fo : fo + 128],
                                bn[:, GOFF + bg * 1024 + ci * 128 : GOFF + bg * 1024 + ci * 128 + 96],
                                start=(cq == 0),
                                stop=False,
                            )
                        for cq in range(4):
                            ci = half * 4 + cq
                            fx = (bg * NCH + ci) * C + ch * 128
                            fy = ((bg + 1) * NCH + ci) * C + ch * 128
                            nc.tensor.matmul(  # X3 (rows 0-31 are zeros)
                                psv[:, 3, :, cq, :],
                                ft[:, fx : fx + 128],
                                bn[:, BOFF + bg * 256 + ci * 32 : BOFF + bg * 256 + ci * 32 + 32],
                                start=False, stop=False,
                            )
                            for pl, off in ((0, COFF), (1, DOFF), (2, EOFF)):
                                r1 = 12 * (pl + 1)
                                nc.tensor.matmul(  # Y0..Y2
                                    psv[:, pl, :, cq, :],
                                    ft[0:r1, fy : fy + 128],
                                    bn[0:r1, off + bg * 256 + ci * 32 : off + bg * 256 + ci * 32 + 32],
                                    start=False, stop=False,
                                )
                            nc.tensor.matmul(  # Y3
                                psv[:, 3, :, cq, :],
                                ft[:, fy : fy + 128],
                                bn[:, GOFF + bg * 1024 + ci * 128 + 96 : GOFF + bg * 1024 + ci * 128 + 128],
                                start=False, stop=(cq == 3),
                            )
                        src = ps.rearrange("p (pl py xx) -> p pl py xx", pl=4, py=2)
                        if (ch + half) % 2 == 0:
                            nc.vector.tensor_copy(stv[:, ch, :, :, half, :], src)
                        else:
                            nc.scalar.copy(stv[:, ch, :, :, half, :], src)
                nc.sync.dma_start(
                    ov[:, :, bg * 1024 : (bg + 1) * 1024],
                    st.rearrange("p (g f) -> p g f", g=2),
                )

    nc.compile()
    _NC_CACHE.append(nc)
    return nc


def kernel(features: np.ndarray, masks: np.ndarray) -> np.ndarray:
    features = np.ascontiguousarray(features, dtype=np.float32)
    masks = np.ascontiguousarray(masks, dtype=np.float32)
    in_maps = _host_prep(features, masks)

    nc = _build_nc()
    res = bass_utils.run_bass_kernel_spmd(nc, in_maps, list(range(NCORES)))

    outv = np.empty((N, C, HO, WO), np.float32)
    for i in range(NCORES):
        n, yh = divmod(i, 2)
        outv[n, :, yh * 64 : (yh + 1) * 64, :] = (
            res.results[i]["out"].astype(np.float32).reshape(C, 64, WO)
        )
    return outv
